# revision 37
# baseline (speedup 1.0000x reference)
"""Multi-head attention forward (B=2, T=2048, C=1024, 16 heads of dim 64)
sharded 8-way tensor-parallel over heads across 8 TRN2 NeuronCores.

Each core computes 2 heads end-to-end:
  qkv^T = w_c^T @ x^T           (weight-stationary, produces transposed layout)
  S^T_h = k_h @ q_h^T           (head-dim contraction; the two heads' K=64
                                 matmuls use disjoint partition ranges so the
                                 PE packs them into one full-rate pass)
  P^T_h = exp(S^T_h)            (no max subtraction: scores are ~N(0,1), |S|<9)
  y^T_h = [v_h | 1]^T @ P^T_h   (ones column yields softmax denominators)
  out_c = sum_h (y_h/denom) @ w_proj[head rows]   (partial projection, bf16)
Host gathers: out = sum_c out_c  (the tensor-parallel all-reduce).

Pipeline layout: ScalarE's exp stream (the second-busiest resource after the
PE) starts as soon as chunk 0 of batch 0 is projected, scores run one
512-token chunk ahead of y/proj, batch-1's qkv projection and the first
chunks' output projections are spread into the exp-bound middle slots so the
PE never idles against the softmax.
"""

import numpy as np
import ml_dtypes
from contextlib import ExitStack

import concourse.bass as bass
import concourse.bacc as bacc
import concourse.mybir as mybir
import concourse.tile as tile
from concourse.bass_utils import run_bass_kernel_spmd
from concourse.masks import make_identity

F32 = mybir.dt.float32
BF16 = mybir.dt.bfloat16
AFT = mybir.ActivationFunctionType

P = 128
NB = 2        # batches
TB = 2048     # tokens per batch
NT = NB * TB  # 4096 tokens total
C = 1024
KC = C // P   # 8 contraction tiles for the qkv projection
QCH = 512     # q-token chunk
NQC = TB // QCH   # 4 q chunks per batch
NKT = TB // P     # 16 k tiles per batch
NCH = NB * NQC    # 8 chunks total
N_CORES = 8
HEAD_DIM = 64
HD1 = HEAD_DIM + 1


def _build_program(nc: bass.Bass):
    xT = nc.declare_dram_parameter("xT", [C, NT], BF16, isOutput=False)[:]
    # wqkv arrives pre-packed partition-major so its single DMA is 128
    # contiguous 6KB descriptors instead of 1024 x 768B
    wqkv = nc.declare_dram_parameter("wqkv", [P, KC, 384], BF16, isOutput=False)[:]
    wproj = nc.declare_dram_parameter("wproj", [2, HEAD_DIM, C], BF16, isOutput=False)[:]
    out = nc.declare_dram_parameter("out", [NT, C], BF16, isOutput=True)[:]

    with tile.TileContext(nc) as tc, ExitStack() as ctx:
        singles = ctx.enter_context(tc.tile_pool(name="singles", bufs=1))
        xin = ctx.enter_context(tc.tile_pool(name="xin", bufs=16))
        vtp = ctx.enter_context(tc.tile_pool(name="vtp", bufs=2))
        ppool = ctx.enter_context(tc.tile_pool(name="ppool", bufs=3))
        small = ctx.enter_context(tc.tile_pool(name="small", bufs=3))
        ybp = ctx.enter_context(tc.tile_pool(name="ybp", bufs=6))
        opool = ctx.enter_context(tc.tile_pool(name="opool", bufs=6))
        psA = ctx.enter_context(tc.tile_pool(name="psA", bufs=2, space="PSUM"))
        pyP = ctx.enter_context(tc.tile_pool(name="pyP", bufs=2, space="PSUM"))
        psB = ctx.enter_context(tc.tile_pool(name="psB", bufs=2, space="PSUM"))

        # ---------------- constants / persistent tensors ----------------
        w_sb = singles.tile([P, KC, 384], BF16, tag="w_sb")
        nc.sync.dma_start(out=w_sb[:], in_=wqkv[:])

        wp_sb = singles.tile([P, C], BF16, tag="wp")
        for h in range(2):
            nc.sync.dma_start(
                out=wp_sb[h * HEAD_DIM : (h + 1) * HEAD_DIM, :], in_=wproj[h]
            )

        ident = singles.tile([P, P], BF16, tag="ident")
        make_identity(nc, ident[:])

        # fmat[h] broadcasts the recip denominator (row 64) to that head's
        # 64-row block of the stacked y tile
        fmat = []
        for h in range(2):
            t = singles.tile([P, P], BF16, tag=f"fmat{h}")
            nc.gpsimd.memset(t[:], 0.0)
            nc.gpsimd.memset(
                t[HEAD_DIM:HD1, h * HEAD_DIM : (h + 1) * HEAD_DIM], 1.0
            )
            fmat.append(t)
        rec = []
        for h in range(2):
            t = singles.tile([P, QCH], BF16, tag=f"rec{h}")
            nc.gpsimd.memset(t[:], 0.0)
            rec.append(t)
        zbias = singles.tile([P, 1], F32, tag="zbias")
        nc.gpsimd.memset(zbias[:], 0.0)

        q_sb = singles.tile([P, NT], BF16, tag="q_sb")
        # k_both rows 0:64 = head0 k dims, rows 64:128 = head1 k dims
        k_both = singles.tile([P, NT], BF16, tag="k_both")
        # v_aug[:, i, h, :] = [v_h for token tile i (64 cols) | ones col]
        v_aug = singles.tile([P, NT // P, 2, HD1], BF16, tag="v_aug")
        nc.vector.memset(v_aug[:, :, :, HEAD_DIM:HD1], 1.0)

        # ---------------- building blocks ----------------
        def xt_load(t):
            tsl = slice(t * QCH, (t + 1) * QCH)
            tiles = []
            for kc in range(KC):
                xt = xin.tile([P, QCH], BF16, tag="xin", name="xt")
                nc.sync.dma_start(out=xt[:], in_=xT[kc * P : (kc + 1) * P, tsl])
                tiles.append(xt)
            return tiles

        def qkv_chain(t, xts, m, dest):
            # one 128-row slice (m=0: q both heads, m=1: k both heads)
            tsl = slice(t * QCH, (t + 1) * QCH)
            ps = psB.tile([P, QCH], F32, tag="psB", name="ps")
            for kc in range(KC):
                nc.tensor.matmul(
                    ps[:],
                    lhsT=w_sb[:, kc, m * P : (m + 1) * P],
                    rhs=xts[kc][:],
                    start=(kc == 0),
                    stop=(kc == KC - 1),
                )
            nc.vector.tensor_copy(out=dest[:, tsl], in_=ps[:])

        def qkv_v_chain(t, xts):
            ps = psB.tile([P, QCH], F32, tag="psB", name="ps")
            for kc in range(KC):
                nc.tensor.matmul(
                    ps[:],
                    lhsT=w_sb[:, kc, 2 * P : 3 * P],
                    rhs=xts[kc][:],
                    start=(kc == 0),
                    stop=(kc == KC - 1),
                )
            vt = vtp.tile([P, QCH], BF16, tag="vt")
            nc.vector.tensor_copy(out=vt[:], in_=ps[:])
            return vt

        def v_transpose(t, vt):
            pt = psB.tile([P, QCH], BF16, tag="psB", name="pt")
            for j in range(QCH // P):
                nc.tensor.transpose(
                    pt[:, j * P : (j + 1) * P], vt[:, j * P : (j + 1) * P], ident[:]
                )
            for j in range(QCH // P):
                i = t * (QCH // P) + j
                nc.vector.tensor_copy(
                    out=v_aug[:, i, 0, 0:HEAD_DIM], in_=pt[:, j * P : j * P + HEAD_DIM]
                )
                nc.vector.tensor_copy(
                    out=v_aug[:, i, 1, 0:HEAD_DIM],
                    in_=pt[:, j * P + HEAD_DIM : (j + 1) * P],
                )

        def sc_kt(b, qc, kt, pT):
            # both heads' scores in one pass: disjoint partition ranges let
            # the PE pack the two K=64 matmuls
            qsl = slice(b * TB + qc * QCH, b * TB + (qc + 1) * QCH)
            ksl = slice(b * TB + kt * P, b * TB + (kt + 1) * P)
            ps = psA.tile([P, 2 * QCH], F32, tag="psA", name="ps")
            nc.tensor.matmul(
                ps[:, 0:QCH],
                lhsT=k_both[0:HEAD_DIM, ksl],
                rhs=q_sb[0:HEAD_DIM, qsl],
                start=True,
                stop=True,
            )
            nc.tensor.matmul(
                ps[:, QCH : 2 * QCH],
                lhsT=k_both[HEAD_DIM:P, ksl],
                rhs=q_sb[HEAD_DIM:P, qsl],
                start=True,
                stop=True,
            )
            nc.scalar.activation(out=pT[:, kt, :, :], in_=ps[:], func=AFT.Exp)

        def pv_chain(b, h, pT, pool=None, tag="pyP"):
            pool = pyP if pool is None else pool
            py = pool.tile([P, QCH], F32, tag=tag, name="py")
            for kt in range(NKT):
                nc.tensor.matmul(
                    py[0:HD1, :],
                    lhsT=v_aug[:, b * NKT + kt, h, :],
                    rhs=pT[:, kt, h, :],
                    start=(kt == 0),
                    stop=(kt == NKT - 1),
                )
            return py

        def norm(pys, pbc_pool=None, pbc_tag="psB"):
            # reciprocal of denominators: 1/d = exp(-ln d) on ScalarE (both
            # funcs live in the same activation table as the main exp stream)
            for h in range(2):
                lnd = small.tile([P, QCH], F32, tag="small", name="lnd")
                nc.scalar.activation(
                    out=lnd[HEAD_DIM:HD1, :],
                    in_=pys[h][HEAD_DIM:HD1, :],
                    func=AFT.Ln,
                    bias=zbias[HEAD_DIM:HD1, :],
                )
                nc.scalar.activation(
                    out=rec[h][HEAD_DIM:HD1, :],
                    in_=lnd[HEAD_DIM:HD1, :],
                    func=AFT.Exp,
                    scale=-1.0,
                    bias=zbias[HEAD_DIM:HD1, :],
                )
            pbc = (psB if pbc_pool is None else pbc_pool).tile(
                [P, QCH], F32, tag=pbc_tag, name="pbc"
            )
            for h in range(2):
                nc.tensor.matmul(
                    pbc[:], lhsT=fmat[h][:], rhs=rec[h][:],
                    start=(h == 0), stop=(h == 1),
                )
            rf = small.tile([P, QCH], F32, tag="small", name="rf")
            nc.vector.tensor_copy(out=rf[:], in_=pbc[:])
            yb = ybp.tile([P, QCH], BF16, tag="yb")
            nc.vector.tensor_mul(
                out=yb[0:HEAD_DIM, :], in0=pys[0][0:HEAD_DIM, :], in1=rf[0:HEAD_DIM, :]
            )
            nc.vector.tensor_mul(
                out=yb[HEAD_DIM:P, :], in0=pys[1][0:HEAD_DIM, :], in1=rf[HEAD_DIM:P, :]
            )
            return yb

        def proj_part(b, qc, yb, tt):
            row0 = b * TB + qc * QCH + tt * P
            for ncol in range(C // QCH):
                po = psB.tile([P, QCH], F32, tag="psB", name="po")
                nc.tensor.matmul(
                    po[:],
                    lhsT=yb[:, tt * P : (tt + 1) * P],
                    rhs=wp_sb[:, ncol * QCH : (ncol + 1) * QCH],
                    start=True,
                    stop=True,
                )
                osb = opool.tile([P, QCH], BF16, tag="osb")
                nc.vector.tensor_copy(out=osb[:], in_=po[:])
                nc.sync.dma_start(
                    out=out[row0 : row0 + P, ncol * QCH : (ncol + 1) * QCH],
                    in_=osb[:],
                )

        chunks = [(b, qc) for b in range(NB) for qc in range(NQC)]
        pTs = {}

        def new_pT():
            return ppool.tile([P, NKT, 2, QCH], BF16, tag="pT", name="pT")

        # -------- phase A: batch-0 qkv with qc0 scores trickled in --------
        xts = xt_load(0)
        qkv_chain(0, xts, 1, k_both)
        qkv_chain(0, xts, 0, q_sb)
        pTs[0] = new_pT()
        for kt in range(0, 4):
            sc_kt(0, 0, kt, pTs[0])
        vt = qkv_v_chain(0, xts)
        v_transpose(0, vt)
        for c in (1, 2, 3):
            xts = xt_load(c)
            qkv_chain(c, xts, 1, k_both)
            for kt in range(4 * c, 4 * c + 4):
                sc_kt(0, 0, kt, pTs[0])
            qkv_chain(c, xts, 0, q_sb)
            vt = qkv_v_chain(c, xts)
            v_transpose(c, vt)

        # -------- main slots: yproj(j) + scores(j+1) + spread-in work --------
        deferred = []  # (b, qc, yb, next_tt) proj work pushed to exp-bound slots
        for j in range(NCH):
            b, qc = chunks[j]
            nxt = j + 1 if j + 1 < NCH else None
            bc = 4 + j if j < 4 else None  # batch-1 qkv handled this slot
            if bc is not None:
                bxts = xt_load(bc)
                qkv_chain(bc, bxts, 1, k_both)
            if nxt is not None:
                pTs[nxt] = new_pT()
                nb_, nqc_ = chunks[nxt]
                for kt in range(NKT):
                    sc_kt(nb_, nqc_, kt, pTs[nxt])
            # drain deferred proj work in the exp-bound slots
            if bc is None:
                for _ in range(6):
                    if deferred:
                        db, dqc, dyb, dtt = deferred[0]
                        proj_part(db, dqc, dyb, dtt)
                        if dtt == 3:
                            deferred.pop(0)
                        else:
                            deferred[0] = (db, dqc, dyb, dtt + 1)
            last = j == NCH - 1
            pys = [
                pv_chain(b, h, pTs[j], pool=psB if last else None,
                         tag="psB" if last else "pyP")
                for h in range(2)
            ]
            if bc is not None:
                qkv_chain(bc, bxts, 0, q_sb)
                bvt = qkv_v_chain(bc, bxts)
                v_transpose(bc, bvt)
            yb = norm(pys, pbc_pool=pyP if last else None,
                      pbc_tag="pyP" if last else "psB")
            if j < 4:
                deferred.append((b, qc, yb, 0))
            else:
                for tt in range(4):
                    proj_part(b, qc, yb, tt)
        while deferred:
            db, dqc, dyb, dtt = deferred.pop(0)
            for tt in range(dtt, 4):
                proj_part(db, dqc, dyb, tt)
    return nc


def _prepare_in_maps(x, w_attn, w_proj):
    bf16 = ml_dtypes.bfloat16
    x = np.asarray(x, dtype=np.float32)
    w_attn = np.asarray(w_attn, dtype=np.float32)
    w_proj = np.asarray(w_proj, dtype=np.float32)

    xT = np.ascontiguousarray(x.reshape(NT, C).T.astype(bf16))  # [C, NT]
    in_maps = []
    for c in range(N_CORES):
        h0, h1 = 2 * c, 2 * c + 1
        cols = []
        for h in (h0, h1):  # q columns, pre-scaled by softmax 1/sqrt(64)
            cols.append(w_attn[:, h * HEAD_DIM : (h + 1) * HEAD_DIM] * 0.125)
        for h in (h0, h1):  # k columns
            cols.append(w_attn[:, C + h * HEAD_DIM : C + (h + 1) * HEAD_DIM])
        for h in (h0, h1):  # v columns
            cols.append(w_attn[:, 2 * C + h * HEAD_DIM : 2 * C + (h + 1) * HEAD_DIM])
        wcat = np.concatenate(cols, axis=1).astype(bf16)  # [C, 384]
        wqkv_c = np.ascontiguousarray(wcat.reshape(KC, P, 384).transpose(1, 0, 2))
        wproj_c = np.ascontiguousarray(
            np.stack(
                [
                    w_proj[h0 * HEAD_DIM : (h0 + 1) * HEAD_DIM, :],
                    w_proj[h1 * HEAD_DIM : (h1 + 1) * HEAD_DIM, :],
                ]
            ).astype(bf16)
        )  # [2, 64, C]
        in_maps.append({"xT": xT, "wqkv": wqkv_c, "wproj": wproj_c})
    return in_maps


class _AttnBacc(bacc.Bacc):
    """Pin all activations to natural_log_exp_and_others so the per-head
    Ln/Exp reciprocal ops don't thrash ACT table loads against the big
    Exp ops."""

    def insert_act_table_loads(self):
        import bass_rust as _bass_rust
        from concourse.hw_specs import get_activation_tables

        has_activation = any(
            isinstance(i, mybir.InstActivation)
            for b in self.main_func.blocks
            for i in b.instructions
        )
        if not has_activation:
            return
        tables = []
        for name, fns in get_activation_tables(self.m.arch).items():
            if name != "natural_log_exp_and_others":
                fns = set()
            tables.append((name, fns))
        _bass_rust.insert_act_table_loads(self, tables)


_CACHED_NC = None


def _get_nc():
    global _CACHED_NC
    if _CACHED_NC is None:
        _CACHED_NC = _build_program(_AttnBacc())
        _CACHED_NC.finalize()
    return _CACHED_NC


def run(x, w_attn, w_proj, trace=False):
    """Returns (output [B, TB, C] float32, BassKernelResults)."""
    in_maps = _prepare_in_maps(x, w_attn, w_proj)
    nc = _get_nc()
    res = run_bass_kernel_spmd(nc, in_maps, core_ids=list(range(N_CORES)), trace=trace)
    acc = np.zeros((NT, C), dtype=np.float32)
    for r in res.results:
        acc += r["out"].astype(np.float32)
    return acc.reshape(NB, TB, C), res


def kernel(x, w_attn, w_proj):
    out, _ = run(x, w_attn, w_proj, trace=False)
    return out


# revision 38
# speedup vs baseline: 1.0165x; 1.0165x over previous
"""Multi-head attention forward (B=2, T=2048, C=1024, 16 heads of dim 64)
sharded 8-way tensor-parallel over heads across 8 TRN2 NeuronCores.

Each core computes 2 heads end-to-end:
  qkv^T = w_c^T @ x^T           (weight-stationary, produces transposed layout)
  S^T_h = k_h @ q_h^T           (head-dim contraction; the two heads' K=64
                                 matmuls use disjoint partition ranges so the
                                 PE packs them into one full-rate pass)
  P^T_h = exp(S^T_h)            (no max subtraction: scores are ~N(0,1), |S|<9)
  y^T_h = [v_h | 1]^T @ P^T_h   (ones column yields softmax denominators)
  out_c = sum_h (y_h/denom) @ w_proj[head rows]   (partial projection, bf16)
Host gathers: out = sum_c out_c  (the tensor-parallel all-reduce).

Pipeline layout: ScalarE's exp stream (the second-busiest resource after the
PE) starts as soon as chunk 0 of batch 0 is projected, scores run one
512-token chunk ahead of y/proj, batch-1's qkv projection and the first
chunks' output projections are spread into the exp-bound middle slots so the
PE never idles against the softmax.
"""

import numpy as np
import ml_dtypes
from contextlib import ExitStack

import concourse.bass as bass
import concourse.bacc as bacc
import concourse.mybir as mybir
import concourse.tile as tile
from concourse.bass_utils import run_bass_kernel_spmd
from concourse.masks import make_identity

F32 = mybir.dt.float32
BF16 = mybir.dt.bfloat16
AFT = mybir.ActivationFunctionType

P = 128
NB = 2        # batches
TB = 2048     # tokens per batch
NT = NB * TB  # 4096 tokens total
C = 1024
KC = C // P   # 8 contraction tiles for the qkv projection
QCH = 512     # q-token chunk
NQC = TB // QCH   # 4 q chunks per batch
NKT = TB // P     # 16 k tiles per batch
NCH = NB * NQC    # 8 chunks total
N_CORES = 8
HEAD_DIM = 64
HD1 = HEAD_DIM + 1


def _build_program(nc: bass.Bass):
    xT = nc.declare_dram_parameter("xT", [C, NT], BF16, isOutput=False)[:]
    # wqkv arrives pre-packed partition-major so its single DMA is 128
    # contiguous 6KB descriptors instead of 1024 x 768B
    wqkv = nc.declare_dram_parameter("wqkv", [P, KC, 384], BF16, isOutput=False)[:]
    wproj = nc.declare_dram_parameter("wproj", [2, HEAD_DIM, C], BF16, isOutput=False)[:]
    out = nc.declare_dram_parameter("out", [NT, C], BF16, isOutput=True)[:]

    with tile.TileContext(nc) as tc, ExitStack() as ctx:
        singles = ctx.enter_context(tc.tile_pool(name="singles", bufs=1))
        xin = ctx.enter_context(tc.tile_pool(name="xin", bufs=16))
        vtp = ctx.enter_context(tc.tile_pool(name="vtp", bufs=2))
        ppool = ctx.enter_context(tc.tile_pool(name="ppool", bufs=3))
        small = ctx.enter_context(tc.tile_pool(name="small", bufs=3))
        ybp = ctx.enter_context(tc.tile_pool(name="ybp", bufs=6))
        opool = ctx.enter_context(tc.tile_pool(name="opool", bufs=6))
        psA = ctx.enter_context(tc.tile_pool(name="psA", bufs=2, space="PSUM"))
        pyP = ctx.enter_context(tc.tile_pool(name="pyP", bufs=2, space="PSUM"))
        psB = ctx.enter_context(tc.tile_pool(name="psB", bufs=2, space="PSUM"))

        # ---------------- constants / persistent tensors ----------------
        w_sb = singles.tile([P, KC, 384], BF16, tag="w_sb")
        nc.sync.dma_start(out=w_sb[:], in_=wqkv[:])

        wp_sb = singles.tile([P, C], BF16, tag="wp")
        for h in range(2):
            nc.sync.dma_start(
                out=wp_sb[h * HEAD_DIM : (h + 1) * HEAD_DIM, :], in_=wproj[h]
            )

        ident = singles.tile([P, P], BF16, tag="ident")
        make_identity(nc, ident[:])

        # fmat[h] broadcasts the recip denominator (row 64) to that head's
        # 64-row block of the stacked y tile
        fmat = []
        for h in range(2):
            t = singles.tile([P, P], BF16, tag=f"fmat{h}")
            nc.gpsimd.memset(t[:], 0.0)
            nc.gpsimd.memset(
                t[HEAD_DIM:HD1, h * HEAD_DIM : (h + 1) * HEAD_DIM], 1.0
            )
            fmat.append(t)
        rec = []
        for h in range(2):
            t = singles.tile([P, QCH], BF16, tag=f"rec{h}")
            nc.gpsimd.memset(t[:], 0.0)
            rec.append(t)
        zbias = singles.tile([P, 1], F32, tag="zbias")
        nc.gpsimd.memset(zbias[:], 0.0)

        q_sb = singles.tile([P, NT], BF16, tag="q_sb")
        # k_both rows 0:64 = head0 k dims, rows 64:128 = head1 k dims
        k_both = singles.tile([P, NT], BF16, tag="k_both")
        # v_aug[:, i, h, :] = [v_h for token tile i (64 cols) | ones col]
        v_aug = singles.tile([P, NT // P, 2, HD1], BF16, tag="v_aug")
        nc.vector.memset(v_aug[:, :, :, HEAD_DIM:HD1], 1.0)

        # ---------------- building blocks ----------------
        def xt_load(t):
            tsl = slice(t * QCH, (t + 1) * QCH)
            tiles = []
            for kc in range(KC):
                xt = xin.tile([P, QCH], BF16, tag="xin", name="xt")
                nc.sync.dma_start(out=xt[:], in_=xT[kc * P : (kc + 1) * P, tsl])
                tiles.append(xt)
            return tiles

        def qkv_chain(t, xts, m, dest):
            # one 128-row slice (m=0: q both heads, m=1: k both heads)
            tsl = slice(t * QCH, (t + 1) * QCH)
            ps = psB.tile([P, QCH], F32, tag="psB", name="ps")
            for kc in range(KC):
                nc.tensor.matmul(
                    ps[:],
                    lhsT=w_sb[:, kc, m * P : (m + 1) * P],
                    rhs=xts[kc][:],
                    start=(kc == 0),
                    stop=(kc == KC - 1),
                )
            nc.vector.tensor_copy(out=dest[:, tsl], in_=ps[:])

        def qkv_v_chain(t, xts):
            ps = psB.tile([P, QCH], F32, tag="psB", name="ps")
            for kc in range(KC):
                nc.tensor.matmul(
                    ps[:],
                    lhsT=w_sb[:, kc, 2 * P : 3 * P],
                    rhs=xts[kc][:],
                    start=(kc == 0),
                    stop=(kc == KC - 1),
                )
            vt = vtp.tile([P, QCH], BF16, tag="vt")
            nc.vector.tensor_copy(out=vt[:], in_=ps[:])
            return vt

        def v_transpose(t, vt):
            pt = psB.tile([P, QCH], BF16, tag="psB", name="pt")
            for j in range(QCH // P):
                nc.tensor.transpose(
                    pt[:, j * P : (j + 1) * P], vt[:, j * P : (j + 1) * P], ident[:]
                )
            for j in range(QCH // P):
                i = t * (QCH // P) + j
                nc.vector.tensor_copy(
                    out=v_aug[:, i, 0, 0:HEAD_DIM], in_=pt[:, j * P : j * P + HEAD_DIM]
                )
                nc.vector.tensor_copy(
                    out=v_aug[:, i, 1, 0:HEAD_DIM],
                    in_=pt[:, j * P + HEAD_DIM : (j + 1) * P],
                )

        def sc_kt(b, qc, kt, pT):
            # both heads' scores in one pass: disjoint partition ranges let
            # the PE pack the two K=64 matmuls
            qsl = slice(b * TB + qc * QCH, b * TB + (qc + 1) * QCH)
            ksl = slice(b * TB + kt * P, b * TB + (kt + 1) * P)
            ps = psA.tile([P, 2 * QCH], F32, tag="psA", name="ps")
            nc.tensor.matmul(
                ps[:, 0:QCH],
                lhsT=k_both[0:HEAD_DIM, ksl],
                rhs=q_sb[0:HEAD_DIM, qsl],
                start=True,
                stop=True,
            )
            nc.tensor.matmul(
                ps[:, QCH : 2 * QCH],
                lhsT=k_both[HEAD_DIM:P, ksl],
                rhs=q_sb[HEAD_DIM:P, qsl],
                start=True,
                stop=True,
            )
            nc.scalar.activation(out=pT[:, kt, :, :], in_=ps[:], func=AFT.Exp)

        def pv_chain(b, h, pT):
            py = pyP.tile([P, QCH], F32, tag="pyP", name="py")
            for kt in range(NKT):
                nc.tensor.matmul(
                    py[0:HD1, :],
                    lhsT=v_aug[:, b * NKT + kt, h, :],
                    rhs=pT[:, kt, h, :],
                    start=(kt == 0),
                    stop=(kt == NKT - 1),
                )
            return py

        def norm(pys):
            # reciprocal of denominators: 1/d = exp(-ln d) on ScalarE (both
            # funcs live in the same activation table as the main exp stream)
            for h in range(2):
                lnd = small.tile([P, QCH], F32, tag="small", name="lnd")
                nc.scalar.activation(
                    out=lnd[HEAD_DIM:HD1, :],
                    in_=pys[h][HEAD_DIM:HD1, :],
                    func=AFT.Ln,
                    bias=zbias[HEAD_DIM:HD1, :],
                )
                nc.scalar.activation(
                    out=rec[h][HEAD_DIM:HD1, :],
                    in_=lnd[HEAD_DIM:HD1, :],
                    func=AFT.Exp,
                    scale=-1.0,
                    bias=zbias[HEAD_DIM:HD1, :],
                )
            pbc = psB.tile([P, QCH], F32, tag="psB", name="pbc")
            for h in range(2):
                nc.tensor.matmul(
                    pbc[:], lhsT=fmat[h][:], rhs=rec[h][:],
                    start=(h == 0), stop=(h == 1),
                )
            rf = small.tile([P, QCH], F32, tag="small", name="rf")
            nc.vector.tensor_copy(out=rf[:], in_=pbc[:])
            yb = ybp.tile([P, QCH], BF16, tag="yb")
            nc.vector.tensor_mul(
                out=yb[0:HEAD_DIM, :], in0=pys[0][0:HEAD_DIM, :], in1=rf[0:HEAD_DIM, :]
            )
            nc.vector.tensor_mul(
                out=yb[HEAD_DIM:P, :], in0=pys[1][0:HEAD_DIM, :], in1=rf[HEAD_DIM:P, :]
            )
            return yb

        def proj_part(b, qc, yb, tt):
            row0 = b * TB + qc * QCH + tt * P
            for ncol in range(C // QCH):
                po = psB.tile([P, QCH], F32, tag="psB", name="po")
                nc.tensor.matmul(
                    po[:],
                    lhsT=yb[:, tt * P : (tt + 1) * P],
                    rhs=wp_sb[:, ncol * QCH : (ncol + 1) * QCH],
                    start=True,
                    stop=True,
                )
                osb = opool.tile([P, QCH], BF16, tag="osb")
                nc.vector.tensor_copy(out=osb[:], in_=po[:])
                nc.sync.dma_start(
                    out=out[row0 : row0 + P, ncol * QCH : (ncol + 1) * QCH],
                    in_=osb[:],
                )

        chunks = [(b, qc) for b in range(NB) for qc in range(NQC)]
        pTs = {}

        def new_pT():
            return ppool.tile([P, NKT, 2, QCH], BF16, tag="pT", name="pT")

        # -------- phase A: batch-0 qkv with qc0 scores trickled in --------
        xts = xt_load(0)
        qkv_chain(0, xts, 1, k_both)
        qkv_chain(0, xts, 0, q_sb)
        pTs[0] = new_pT()
        for kt in range(0, 4):
            sc_kt(0, 0, kt, pTs[0])
        vt = qkv_v_chain(0, xts)
        v_transpose(0, vt)
        for c in (1, 2, 3):
            xts = xt_load(c)
            qkv_chain(c, xts, 1, k_both)
            for kt in range(4 * c, 4 * c + 4):
                sc_kt(0, 0, kt, pTs[0])
            qkv_chain(c, xts, 0, q_sb)
            vt = qkv_v_chain(c, xts)
            v_transpose(c, vt)

        # -------- main slots: yproj(j) + scores(j+1) + spread-in work --------
        deferred = []  # (b, qc, yb, next_tt) proj work pushed to exp-bound slots
        for j in range(NCH):
            b, qc = chunks[j]
            nxt = j + 1 if j + 1 < NCH else None
            bc = 4 + j if j < 4 else None  # batch-1 qkv handled this slot
            if bc is not None:
                bxts = xt_load(bc)
                qkv_chain(bc, bxts, 1, k_both)
            if nxt is not None:
                pTs[nxt] = new_pT()
                nb_, nqc_ = chunks[nxt]
                for kt in range(NKT):
                    sc_kt(nb_, nqc_, kt, pTs[nxt])
            # drain deferred proj work in the exp-bound slots
            if bc is None:
                for _ in range(6):
                    if deferred:
                        db, dqc, dyb, dtt = deferred[0]
                        proj_part(db, dqc, dyb, dtt)
                        if dtt == 3:
                            deferred.pop(0)
                        else:
                            deferred[0] = (db, dqc, dyb, dtt + 1)
            pys = [pv_chain(b, h, pTs[j]) for h in range(2)]
            if bc is not None:
                qkv_chain(bc, bxts, 0, q_sb)
                bvt = qkv_v_chain(bc, bxts)
                v_transpose(bc, bvt)
            yb = norm(pys)
            if j < 4:
                deferred.append((b, qc, yb, 0))
            else:
                for tt in range(4):
                    proj_part(b, qc, yb, tt)
        while deferred:
            db, dqc, dyb, dtt = deferred.pop(0)
            for tt in range(dtt, 4):
                proj_part(db, dqc, dyb, tt)
    return nc


def _prepare_in_maps(x, w_attn, w_proj):
    bf16 = ml_dtypes.bfloat16
    x = np.asarray(x, dtype=np.float32)
    w_attn = np.asarray(w_attn, dtype=np.float32)
    w_proj = np.asarray(w_proj, dtype=np.float32)

    xT = np.ascontiguousarray(x.reshape(NT, C).T.astype(bf16))  # [C, NT]
    in_maps = []
    for c in range(N_CORES):
        h0, h1 = 2 * c, 2 * c + 1
        cols = []
        for h in (h0, h1):  # q columns, pre-scaled by softmax 1/sqrt(64)
            cols.append(w_attn[:, h * HEAD_DIM : (h + 1) * HEAD_DIM] * 0.125)
        for h in (h0, h1):  # k columns
            cols.append(w_attn[:, C + h * HEAD_DIM : C + (h + 1) * HEAD_DIM])
        for h in (h0, h1):  # v columns
            cols.append(w_attn[:, 2 * C + h * HEAD_DIM : 2 * C + (h + 1) * HEAD_DIM])
        wcat = np.concatenate(cols, axis=1).astype(bf16)  # [C, 384]
        wqkv_c = np.ascontiguousarray(wcat.reshape(KC, P, 384).transpose(1, 0, 2))
        wproj_c = np.ascontiguousarray(
            np.stack(
                [
                    w_proj[h0 * HEAD_DIM : (h0 + 1) * HEAD_DIM, :],
                    w_proj[h1 * HEAD_DIM : (h1 + 1) * HEAD_DIM, :],
                ]
            ).astype(bf16)
        )  # [2, 64, C]
        in_maps.append({"xT": xT, "wqkv": wqkv_c, "wproj": wproj_c})
    return in_maps


class _AttnBacc(bacc.Bacc):
    """Pin all activations to natural_log_exp_and_others so the per-head
    Ln/Exp reciprocal ops don't thrash ACT table loads against the big
    Exp ops."""

    def insert_act_table_loads(self):
        import bass_rust as _bass_rust
        from concourse.hw_specs import get_activation_tables

        has_activation = any(
            isinstance(i, mybir.InstActivation)
            for b in self.main_func.blocks
            for i in b.instructions
        )
        if not has_activation:
            return
        tables = []
        for name, fns in get_activation_tables(self.m.arch).items():
            if name != "natural_log_exp_and_others":
                fns = set()
            tables.append((name, fns))
        _bass_rust.insert_act_table_loads(self, tables)


_CACHED_NC = None


def _get_nc():
    global _CACHED_NC
    if _CACHED_NC is None:
        _CACHED_NC = _build_program(_AttnBacc())
        _CACHED_NC.finalize()
    return _CACHED_NC


def run(x, w_attn, w_proj, trace=False):
    """Returns (output [B, TB, C] float32, BassKernelResults)."""
    in_maps = _prepare_in_maps(x, w_attn, w_proj)
    nc = _get_nc()
    res = run_bass_kernel_spmd(nc, in_maps, core_ids=list(range(N_CORES)), trace=trace)
    acc = np.zeros((NT, C), dtype=np.float32)
    for r in res.results:
        acc += r["out"].astype(np.float32)
    return acc.reshape(NB, TB, C), res


def kernel(x, w_attn, w_proj):
    out, _ = run(x, w_attn, w_proj, trace=False)
    return out
